# revision 87
# baseline (speedup 1.0000x reference)
"""Trainium2 Bass kernel for nn_Attention: 16-head attention, B=2, S=2048, H=1024.

Megatron-style tensor parallel over heads: 8 cores x 2 heads. Host sums the 8
partial dense outputs (all-reduce-after-dense recipe) and applies the bias
terms that commute out of the kernel.

Per-core dataflow (all matmul inputs bf16, fp32 PSUM accumulation):
  - q,k computed in [dim, token] layout (moving = x^T blocks, ap=512).
  - v computed directly in [token, dim] layout (stationary = x^T chunk,
    moving = v-weights), so no PE transposes are needed for v.
  - scores^T: PSUM [128 keys, 1024] holds two key-chunks x 512 queries; exp
    runs as one [128,1024] instruction, split between the Act engine (Exp
    activation, scale=1/8) and the DVE (pow with constant base e^{1/8}).
  - ctx accumulated in [token, dim] orientation: stationary = probs chunk,
    moving = v chunk with a ones column appended (65th column accumulates the
    softmax denominator for free).
  - late normalization (DVE reciprocal + per-partition scale), PE transpose of
    the normalized ctx to [dim, token], dense with moving = dense weights.
  - engine split: PE matmuls; Act = exp + q/k psum->sbuf (q-bias fused);
    DVE = exp + normalize + reciprocal; Pool = v/ctxT/dense-out copies + DMA
    queues for the streamed x^T blocks and output tiles.
  - bias handling: k-bias is softmax-invariant (dropped), v-bias and dense
    bias are added on the host, q-bias is fused into the q PSUM->SBUF copy.
"""
import math
import os

import numpy as np
import ml_dtypes

B, S, H, NH = 2, 2048, 1024, 16
HD = H // NH             # 64
BS = B * S               # 4096
NCORES = 8
NKK = H // 128           # 8 contraction chunks
NBLK = BS // 512         # 8 token blocks of 512
NQB = S // 512           # 4 query blocks per batch
NKC = S // 128           # 16 key chunks per batch
NPAIR = NKC // 2         # 8 key-chunk pairs per query block
NTC = BS // 128          # 32 token chunks of 128

_CACHE = {}

EXP_BASE = float(np.exp(0.125))  # e^{1/8}; (e^{1/8})^s == exp(s/8)


def _build_program():
    import concourse.mybir as mybir
    import concourse.tile as tile
    from concourse import bacc

    F32 = mybir.dt.float32
    F32R = mybir.dt.float32r
    BF16 = mybir.dt.bfloat16
    FP8 = mybir.dt.float8e4
    DR = mybir.MatmulPerfMode.DoubleRow
    Act = mybir.ActivationFunctionType
    Alu = mybir.AluOpType

    nc = bacc.Bacc("TRN2", target_bir_lowering=False, debug=False,
                   num_devices=NCORES)
    xtb = nc.dram_tensor("xtb", [H, BS], BF16, kind="ExternalInput").ap()
    wq = nc.dram_tensor("wq", [128, NKK, 128], BF16, kind="ExternalInput").ap()
    wk = nc.dram_tensor("wk", [128, NKK, 128], BF16, kind="ExternalInput").ap()
    wv = nc.dram_tensor("wv", [128, NKK, 128], BF16, kind="ExternalInput").ap()
    w2m = nc.dram_tensor("w2m", [128, H], BF16, kind="ExternalInput").ap()
    qbias = nc.dram_tensor("qbias", [128, 1], F32, kind="ExternalInput").ap()
    ident = nc.dram_tensor("ident", [128, 128], BF16, kind="ExternalInput").ap()
    out = nc.dram_tensor("out", [BS, H], BF16, kind="ExternalOutput").ap()
    dbg = {}
    if os.environ.get("KDBG"):
        dbg["qsb"] = nc.dram_tensor("dbg_qsb", [128, BS], F32, kind="ExternalOutput").ap()
        dbg["ksb"] = nc.dram_tensor("dbg_ksb", [128, BS], F32, kind="ExternalOutput").ap()
        dbg["vsb"] = nc.dram_tensor("dbg_vsb", [128, NTC * 130], F32, kind="ExternalOutput").ap()
        dbg["pt"] = nc.dram_tensor("dbg_pt", [128, 1024], F32, kind="ExternalOutput").ap()
        dbg["ctxa"] = nc.dram_tensor("dbg_ctxa", [128, NQB * 65], F32, kind="ExternalOutput").ap()
        dbg["rec"] = nc.dram_tensor("dbg_rec", [128, NQB], F32, kind="ExternalOutput").ap()
        dbg["ctxt"] = nc.dram_tensor("dbg_ctxt", [128, 16 * 128], F32, kind="ExternalOutput").ap()

    with tile.TileContext(nc) as tc, nc.allow_low_precision(reason="bf16"):
        from contextlib import ExitStack
        with ExitStack() as ctx:
            consts = ctx.enter_context(tc.tile_pool(name="consts", bufs=1))
            persist = ctx.enter_context(tc.tile_pool(name="persist", bufs=1))
            xtp = ctx.enter_context(tc.tile_pool(name="xtp", bufs=8))
            fp8t = ctx.enter_context(tc.tile_pool(name="fp8t", bufs=4))
            probs = ctx.enter_context(tc.tile_pool(name="probs", bufs=12))
            recipp = ctx.enter_context(tc.tile_pool(name="recipp", bufs=4))
            ctxnp = ctx.enter_context(tc.tile_pool(name="ctxnp", bufs=12))
            ctxtp = ctx.enter_context(tc.tile_pool(name="ctxtp", bufs=2))
            ostage = ctx.enter_context(tc.tile_pool(name="ostage", bufs=4))
            ps_sc = ctx.enter_context(
                tc.tile_pool(name="ps_sc", bufs=2, space="PSUM"))
            ps_ctx = ctx.enter_context(
                tc.tile_pool(name="ps_ctx", bufs=2, space="PSUM"))
            ps_qd = ctx.enter_context(
                tc.tile_pool(name="ps_qd", bufs=2, space="PSUM"))

            # ---- constants ----
            # DGE issue order is DMA transfer order: wq, first half of x^T
            # block 0, wk, second half, so q and k matmuls of block 0 can
            # start after ~1.4MB of DMA. Weight tiles go as single coalesced
            # DMAs (2KB contiguous per partition = full DMA descriptor rate).
            wqsb = consts.tile([128, NKK, 128], BF16, name="wqsb")
            xt_t0 = xtp.tile([128, NKK, 512], BF16, name="xt")
            src0 = xtb[:, 0:512].rearrange("(k p) t -> p k t", p=128)
            nc.sync.dma_start(wqsb[:], wq)
            nc.sync.dma_start(xt_t0[:, 0:NKK // 2, :], src0[:, 0:NKK // 2, :])
            wksb = consts.tile([128, NKK, 128], BF16, name="wksb")
            nc.sync.dma_start(wksb[:], wk)
            nc.sync.dma_start(xt_t0[:, NKK // 2:, :], src0[:, NKK // 2:, :])
            qbsb = consts.tile([128, 1], F32, name="qbsb")
            nc.sync.dma_start(qbsb[:], qbias)
            warm = consts.tile([1, 1], F32, name="warm")
            nc.scalar.activation(warm[0:1, 0:1], qbsb[0:1, 0:1], Act.Exp)
            # PE p-state warm-up: the cost of the first ~3us of PE work is
            # 2-4x while the clock ramps. Burn the DMA-gated startup window
            # on throwaway matmuls (memset operands, scratch PSUM) so block
            # 0's real qkv runs at full clock.
            wrm = consts.tile([128, 64], BF16, name="wrm")
            nc.vector.memset(wrm[:], 0.0)
            wps = ps_qd.tile([128, 64], F32, name="wps", tag="qd",
                             padded_shape=[128, 512])

            def pe_warm(k):
                for _ in range(k):
                    nc.tensor.matmul(wps[0:64, :], wrm[:], wrm[:],
                                     start=True, stop=True)

            pe_warm(72)

            # ---- persistent q/k/v ----
            # q/k live as fp8e4m3 hi/lo DoubleRow stacks, one per head:
            #   qsd[j][p, n, t]: partitions [q_hi(64); q_lo(64)] of block n
            #   ksd[j][p, n, pl, t]: plane0 [k_hi; k_lo], plane1 [k_lo; k_hi]
            # One DoubleRow matmul then computes (q_hi+q_lo)(k_hi+k_lo)
            # exactly -- bf16-grade precision at 2x PE throughput.
            qsd = [persist.tile([128, NBLK, 512], FP8, name=f"qsd{j}")
                   for j in range(2)]
            ksd = [persist.tile([128, NBLK, 2, 512], FP8, name=f"ksd{j}")
                   for j in range(2)]
            # bf16 q/k mirror for the ramp: qb0's scores run straight off
            # the qkv drains (no restack latency) while the fp8 stacks for
            # the remaining 28 attention blocks build in the background.
            qsb0 = persist.tile([128, NQB, 512], BF16, name="qsb0")
            ksb0 = persist.tile([128, NQB, 512], BF16, name="ksb0")
            # v layout: [token-part, chunk, 2*(64+1)]; cols 64 and 129 hold
            # the ones column that accumulates the softmax denominator.
            vsb = persist.tile([128, NTC, 130], BF16, name="vsb")
            # only the ones-columns (64, 129) need initialization -- the
            # data columns are fully written by the v drains. Keeps Pool
            # free for the block-0 k-restacks during the DMA ramp.
            nc.gpsimd.memset(vsb[:, :, 64:65], 1.0)
            nc.gpsimd.memset(vsb[:, :, 129:130], 1.0)

            late_consts = {}

            # ---- phase A: qkv projection for one 512-token block ----
            # Emitted as a list of small closures ("groups", ~0.6us of PE
            # work each) so blocks 4-7 can interleave into batch-0 attention
            # pairs as exp-independent PE filler.
            xts = {}

            def dma_block(n):
                if n == 0:
                    xts[n] = xt_t0
                    return
                xt_t = xtp.tile([128, NKK, 512], BF16, name="xt")
                src = xtb[:, n * 512:(n + 1) * 512].rearrange(
                    "(k p) t -> p k t", p=128)
                nc.sync.dma_start(xt_t[:], src)
                xts[n] = xt_t

            def qkv_groups(n):
                state = {"xt": xts[n], "defer": []}

                def g_start():
                    xt_t = state["xt"]
                    qps = ps_qd.tile([128, 512], F32, name="qps", tag="qd")
                    state["qps"] = qps
                    for kk in range(NKK // 2):
                        nc.tensor.matmul(qps[:], wqsb[:, kk, :], xt_t[:, kk, :],
                                         start=(kk == 0), stop=False)

                def q_fp8():
                    qps = state["qps"]
                    hi = fp8t.tile([128, 512], FP8, name="qhi")
                    nc.vector.tensor_scalar_add(hi[:], qps[:], qbsb[:, 0:1])
                    lo = fp8t.tile([128, 512], FP8, name="qlo")
                    nc.vector.scalar_tensor_tensor(
                        lo[:], qps[:], qbsb[:, 0:1], hi[:],
                        Alu.add, Alu.subtract)
                    for j in range(2):
                        nc.gpsimd.tensor_copy(qsd[j][0:64, n, :],
                                              hi[j * 64:j * 64 + 64, :])
                        nc.gpsimd.tensor_copy(qsd[j][64:128, n, :],
                                              lo[j * 64:j * 64 + 64, :])

                def g_q2():
                    xt_t, qps = state["xt"], state["qps"]
                    for kk in range(NKK // 2, NKK):
                        nc.tensor.matmul(qps[:], wqsb[:, kk, :], xt_t[:, kk, :],
                                         start=False, stop=(kk == NKK - 1))
                    if n < NQB:
                        # batch 0 scores run bf16 straight off the drains
                        # (PE has headroom under the Act exp wall); the fp8
                        # DoubleRow stacks are only built for batch 1.
                        nc.vector.tensor_scalar_add(qsb0[:, n, :], qps[:],
                                                    qbsb[:, 0:1])
                        return
                    q_fp8()

                def g_k1():
                    xt_t = state["xt"]
                    kps = ps_qd.tile([128, 512], F32, name="kps", tag="qd")
                    state["kps"] = kps
                    for kk in range(NKK // 2):
                        nc.tensor.matmul(kps[:], wksb[:, kk, :], xt_t[:, kk, :],
                                         start=(kk == 0), stop=False)

                def k_fp8():
                    kps = state["kps"]
                    hi = fp8t.tile([128, 512], FP8, name="khi")
                    nc.vector.tensor_scalar_add(hi[:], kps[:], 0.0)
                    lo = fp8t.tile([128, 512], FP8, name="klo")
                    nc.vector.scalar_tensor_tensor(
                        lo[:], kps[:], 0.0, hi[:], Alu.add, Alu.subtract)
                    # head 0's stack first: its scores run one exp-slot
                    # before head 1's, so Pool's restack latency hides.
                    for j in range(2):
                        hj = hi[j * 64:j * 64 + 64, :]
                        lj = lo[j * 64:j * 64 + 64, :]
                        nc.gpsimd.tensor_copy(ksd[j][0:64, n, 0, :], hj)
                        nc.gpsimd.tensor_copy(ksd[j][64:128, n, 0, :], lj)
                        nc.gpsimd.tensor_copy(ksd[j][0:64, n, 1, :], lj)
                        nc.gpsimd.tensor_copy(ksd[j][64:128, n, 1, :], hj)

                def g_k2():
                    xt_t, kps = state["xt"], state["kps"]
                    for kk in range(NKK // 2, NKK):
                        nc.tensor.matmul(kps[:], wksb[:, kk, :], xt_t[:, kk, :],
                                         start=False, stop=(kk == NKK - 1))
                    if n < NQB:
                        nc.vector.tensor_copy(ksb0[:, n, :], kps[:])
                        return
                    k_fp8()

                def g_v(t4):
                    xt_t = state["xt"]
                    if t4 == 0:
                        state["vps"] = ps_qd.tile([128, 4, 128], F32,
                                                  name="vps", tag="qd")
                    vps = state["vps"]
                    for kk in range(NKK):
                        nc.tensor.matmul(
                            vps[:, t4, :],
                            xt_t[:, kk, t4 * 128:(t4 + 1) * 128],
                            late_consts["wvsb"][:, kk, :],
                            start=(kk == 0 and t4 == 0),
                            stop=(kk == NKK - 1 and t4 == 3))
                    if t4 == 3:
                        for u4 in range(4):
                            g = n * 4 + u4
                            nc.vector.tensor_copy(
                                vsb[:, g, :].rearrange("p (j w) -> p j w",
                                                       w=65)[:, :, 0:64],
                                vps[:, u4, :].rearrange("p (j w) -> p j w",
                                                        w=64))

                # (cost_ns, fn) in consumption-deadline order: k feeds the
                # scores of every batch-b qb block (key chunk kc reads block
                # kc//4), v feeds ctx two pairs later, q only feeds qb block
                # n's own scores.
                kk_ = [(852, g_k1), (852, g_k2)]
                vv = [(427, lambda: g_v(0)), (427, lambda: g_v(1)),
                      (427, lambda: g_v(2)), (427, lambda: g_v(3))]
                qq = [(852, g_start), (852, g_q2)]
                return kk_, vv, qq, state

            def dbg_dump(name, ap_src, cols):
                if not dbg:
                    return
                st = ostage.tile([128, H], F32, name="dbgst")
                nc.vector.tensor_copy(st[:, 0:cols], ap_src)
                nc.sync.dma_start(dbg[name][:, 0:cols], st[:, 0:cols])

            # ---- phase B helpers ----
            def emit_norm(pend):
                """Reciprocal of the denominators (read straight out of the
                PSUM ones-column) + ONE unnormalized ctx copy to SBUF; the
                softmax normalization is applied later as a per-token scale
                on the dense PSUM->SBUF drain. Keeps the cross-block ctxa
                WAR latency to two short DVE ops."""
                b, j, qb, ctxa, ctxt = pend
                rec = recipp.tile([128, NQB], F32, name="rec", tag="rec")
                nc.vector.reciprocal_approx_fast(rec[:], ctxa[:, :, 64])
                if dbg and (b, j, qb) == (0, 0, 0):
                    dbg_dump("rec", rec[:], NQB)
                cn = ctxnp.tile([128, NQB, 64], BF16, name="cn")
                # normalized ctx in ONE DVE op: rec broadcast along the
                # 64-wide dim via a 0-stride read.
                nc.vector.scalar_tensor_tensor(
                    cn[:], ctxa[:, :, 0:64], 1.0,
                    rec[:].broadcast_to([128, NQB, 64]),
                    Alu.bypass, Alu.mult)
                return [rec, cn]

            def emit_tp(pend, cns):
                """Transpose (unnormalized) ctx to [dim, token]."""
                b, j, qb, ctxa, ctxt = pend
                cn = cns[1]
                tp = ps_qd.tile([128, NQB, 128], BF16, name="tp", tag="qd",
                                padded_shape=[128, NQB, 256])
                for qc in range(NQB):
                    nc.tensor.matmul(tp[0:64, qc, :],
                                     cn[:, qc, :],
                                     late_consts["idsb"][:],
                                     is_transpose=True,
                                     start=(qc == 0), stop=(qc == NQB - 1))
                for qc in range(NQB):
                    t = qb * 4 + qc
                    nc.vector.tensor_copy(
                        ctxt[j * 64:(j + 1) * 64, t, :],
                        tp[0:64, qc, :])

            def emit_dense_qc(pend, qc, recs, tail=False):
                """Dense partial for one 128-token chunk of pend (j==1):
                ctxt already holds normalized ctx for both heads. Tail jobs
                drain on the (by then idle) Act engine and DMA per half so
                the final chain is not DVE-paced."""
                b, j, qb, ctxa, ctxt = pend
                t = qb * 4 + qc
                od = ostage.tile([128, H], BF16, name="od")
                row0 = b * S + t * 128
                for nb in range(2):
                    dp = ps_qd.tile([128, 512], F32, name="dp", tag="qd")
                    nc.tensor.matmul(
                        dp[:], ctxt[:, t, :],
                        late_consts["w2sb"][:, nb * 512:(nb + 1) * 512],
                        start=True, stop=True)
                    if tail and nb == 0:
                        nc.scalar.activation(
                            od[:, nb * 512:(nb + 1) * 512], dp[:],
                            Act.Identity)
                    else:
                        nc.vector.tensor_copy(
                            od[:, nb * 512:(nb + 1) * 512], dp[:])
                    if tail:
                        nc.sync.dma_start(
                            out[row0:row0 + 128, nb * 512:(nb + 1) * 512],
                            od[:, nb * 512:(nb + 1) * 512])
                if not tail:
                    nc.sync.dma_start(out[row0:row0 + 128, :], od[:])

            def emit_score(sp, hf, b, j, qb, kc):
                """One key-chunk of scores. Batch 0: bf16 off the qkv
                drains (PE headroom, zero restack latency). Batch 1: fp8
                hi/lo DoubleRow at 2x PE rate off the Pool-built stacks."""
                if b == 0:
                    nc.tensor.matmul(
                        sp[:, hf * 512:(hf + 1) * 512],
                        ksb0[j * 64:(j + 1) * 64, kc // 4,
                             (kc % 4) * 128:(kc % 4 + 1) * 128],
                        qsb0[j * 64:(j + 1) * 64, qb, :],
                        start=True, stop=True)
                    return
                blk = b * NQB + kc // 4
                c0 = (kc % 4) * 128
                rhs = qsd[j][:, b * NQB + qb, :].rearrange(
                    "p (o t) -> p o t", o=1).broadcast_to([128, 2, 512])
                nc.tensor.matmul(
                    sp[:, hf * 512:(hf + 1) * 512],
                    ksd[j][:, blk, :, c0:c0 + 128], rhs,
                    start=True, stop=True, perf_mode=DR)

            dense_jobs = []    # (cost, fn) dense chunks waiting for a slot
            fill_jobs = []     # (cost, fn, tag) qkv groups as PE filler
            done_tags = set()
            deficit = [0.0]

            def require(tag):
                """Force-emit fill jobs up to and including `tag` so every
                attention instruction's qkv producers are already emitted
                (the tile framework can only order already-emitted work)."""
                if tag in done_tags:
                    return
                while fill_jobs:
                    c, fn, t = fill_jobs.pop(0)
                    fn()
                    done_tags.add(t)
                    if t == tag:
                        return

            def pump(budget_ns):
                """Deficit-based filler: run queued dense/qkv jobs until the
                accumulated budget is spent. Dense first (it drains PSUM and
                feeds the output DMA pipeline)."""
                deficit[0] += budget_ns
                while deficit[0] > 0 and (dense_jobs or fill_jobs):
                    if dense_jobs:
                        c, fn = dense_jobs.pop(0)
                    else:
                        c, fn, t = fill_jobs.pop(0)
                        done_tags.add(t)
                    fn()
                    deficit[0] -= c

            def emit_attention_qb(b, j, qb, ctxt, pend, fill_ns):
                """One 512-query block of head j, batch b. Returns new pend."""
                ctxa = ps_ctx.tile([128, NQB, 65], F32, name="ctxa",
                                   padded_shape=[128, NQB, 128])
                pts = {}
                for pr in range(NPAIR):
                    if pr == 0:
                        require(("q", b * NQB + qb))
                    require(("k", b * NQB + pr // 2))
                    sp = ps_sc.tile([128, 1024], F32, name="sp")
                    for hf in range(2):
                        kc = pr * 2 + hf
                        emit_score(sp, hf, b, j, qb, kc)
                    # exp runs on Act only: the DVE datapath has no exp
                    # and GPSIMD cannot read PSUM on TRN2.
                    pt = probs.tile([128, 1024], BF16, name="pt")
                    nc.scalar.activation(pt[:], sp[:], Act.Exp, scale=0.125)
                    pts[pr] = pt
                    # Filler AFTER the pair's scores+exp: a fill job stalled
                    # on DMA must never sit ahead of ready attention work in
                    # PE program order.
                    # Lookahead: force the next blocks' producers while this
                    # block's exps keep Act busy, so forced qkv bursts hide
                    # under attention instead of stalling it.
                    nxt = b * NQB + min(NQB - 1, pr // 2 + 1)
                    require(("k", nxt))
                    require(("v", nxt))
                    if j == 1:
                        if qb < NQB - 1 and pr == 3:
                            require(("q", b * NQB + qb + 1))
                        if b == 0 and qb >= NQB - 2:
                            st = (qb - (NQB - 2)) * 2
                            if pr == 1:
                                require(("k", NQB + st))
                            if pr == 5:
                                if qb == NQB - 2:
                                    require(("q", NQB))
                                require(("k", NQB + st + 1))
                            if qb == NQB - 1 and pr == 3:
                                require(("q", NQB))
                    if pr % 2 == 1 and dense_jobs:
                        c, fn = dense_jobs.pop(0)
                        fn()
                        deficit[0] -= c
                    pump(fill_ns[pr])
                    if dbg and (b, j, qb, pr) == (0, 0, 0, 0):
                        dbg_dump("pt", pt[:], 1024)
                    if pr == 2 and pend is not None:
                        emit_tp(pend[:5], pend[5])
                        # pend's ctxt region is now written: its dense work
                        # may enter the filler queue (never before, or dense
                        # would read a not-yet-emitted transpose).
                        if pend[1] == 1:
                            for qc in range(NQB):
                                dense_jobs.append(
                                    (426, lambda p=pend[:5], q=qc,
                                     r=pend[5][0]: emit_dense_qc(p, q, r)))
                    if pr >= 2:
                        require(("v", b * NQB + (pr - 2) // 2))
                        emit_ctx(b, j, pr - 2, ctxa, pts.pop(pr - 2))
                require(("v", b * NQB + NQB - 1))
                emit_ctx(b, j, NPAIR - 2, ctxa, pts.pop(NPAIR - 2))
                emit_ctx(b, j, NPAIR - 1, ctxa, pts.pop(NPAIR - 1))
                cur = (b, j, qb, ctxa, ctxt)
                if dbg and (b, j, qb) == (0, 0, 0):
                    dbg_dump("ctxa", ctxa[:].rearrange("p a b -> p (a b)"), NQB * 65)
                cns = emit_norm(cur)
                return cur + (cns,)

            def emit_ctx(b, j, pr, ctxa, pt):
                # a start=True matmul zeroes the whole 2KB PSUM bank, so the
                # four qc sub-accumulators chain into ONE group: only the
                # first matmul starts it, only the last stops it.
                for hf in range(2):
                    kc = pr * 2 + hf
                    for qc in range(NQB):
                        nc.tensor.matmul(
                            ctxa[:, qc, :],
                            pt[:, hf * 512 + qc * 128:hf * 512 + (qc + 1) * 128],
                            vsb[:, b * 16 + kc, j * 65:(j + 1) * 65],
                            start=(kc == 0 and qc == 0),
                            stop=(kc == NKC - 1 and qc == NQB - 1))

            # ---- emission schedule ----
            # DGE order = transfer order: wv right after block 0's x so the
            # inline v-groups of block 0 don't stall; ident before the first
            # transpose (~pair 18); w2m before the first dense (~pair 16);
            # the batch-1 x blocks (4-7) last.
            dma_block(0)
            kk0, vv0, qq0, st0 = qkv_groups(0)
            # k1, q1, k2, q2: both bf16 drains land as early as possible;
            # the fp8 stack builds and block-0 v are deferred off the
            # first-exp critical path (queued at the fill front). Warm-up
            # matmuls keep the PE clock ramped across the DMA-gated gaps.
            kk0[0][1]()
            pe_warm(16)
            qq0[0][1]()
            pe_warm(16)
            kk0[1][1]()
            qq0[1][1]()
            pe_warm(16)
            dma_block(1)
            wvsb = consts.tile([128, NKK, 128], BF16, name="wvsb")
            nc.sync.dma_start(wvsb[:], wv)
            late_consts["wvsb"] = wvsb
            idsb = consts.tile([128, 128], BF16, name="idsb")
            nc.sync.dma_start(idsb[:], ident)
            late_consts["idsb"] = idsb
            dma_block(2)
            dma_block(3)
            w2sb = consts.tile([128, H], BF16, name="w2sb")
            nc.sync.dma_start(w2sb[:], w2m)
            late_consts["w2sb"] = w2sb
            for n in range(4, NBLK):
                dma_block(n)
            # Fill queue in consumption-deadline order: deferred block-0
            # fp8/v first (fp8 before v: the deferred drains must re-read
            # the k/q PSUMs before the v matmuls recycle those buffers),
            # then k/v of blocks 1-3 (batch-0 scores/ctx), q of 1-3, then
            # batch 1's blocks.
            gq = {}
            for n in range(1, NBLK):
                kk_, vv, qq, _st = qkv_groups(n)
                kk_ = [(c, f, ("k", n) if i == len(kk_) - 1 else None)
                       for i, (c, f) in enumerate(kk_)]
                vv = [(c, f, ("v", n) if i == len(vv) - 1 else None)
                      for i, (c, f) in enumerate(vv)]
                qq = [(c, f, ("q", n) if i == len(qq) - 1 else None)
                      for i, (c, f) in enumerate(qq)]
                gq[n] = (kk_, vv, qq)
            done_tags.update([("k", 0), ("q", 0)])
            fill_jobs.extend(
                (c, f, ("v", 0) if i == len(vv0) - 1 else None)
                for i, (c, f) in enumerate(vv0))
            for n in (1, 2, 3):
                fill_jobs.extend(gq[n][0])
                fill_jobs.extend(gq[n][1])
            fill_jobs.extend(gq[1][2])
            fill_jobs.extend(gq[2][2])
            fill_jobs.extend(gq[3][2])
            fill_jobs.extend(gq[4][0])
            fill_jobs.extend(gq[4][1])
            fill_jobs.extend(gq[4][2])
            for n in range(5, NBLK):
                fill_jobs.extend(gq[n][0])
                fill_jobs.extend(gq[n][1])
            for n in range(5, NBLK):
                fill_jobs.extend(gq[n][2])

            # Per-pair filler budget (ns of PE work). Steady state is
            # total_filler/128 ~= 390ns so PE stays ~99% fed at the Act exp
            # cadence (~1038ns/pair).
            STEADY = [390] * NPAIR
            RAMP = [0, 1200, 1600, 2000, 2000, 2000, 2000, 2000]
            pend = None
            ctxts = {0: ctxtp.tile([128, S // 128, 128], BF16, name="ctxt0")}

            def emit_qb0_interleaved(ctxt):
                """Startup special case: the first 16 pairs are DMA-gated by
                the x^T stream (k/v blocks 1-3 arrive ~3.5us apart). Head 0
                and head 1 of (b0, qb0) share every k-chunk, so interleave
                their scores+exp streams pairwise -- two exps per arriving
                chunk instead of one -- and hold head 1's probs in SBUF
                until head 0's ctxa frees (single PSUM accumulator)."""
                ctxa0 = ps_ctx.tile([128, NQB, 65], F32, name="ctxa",
                                    padded_shape=[128, NQB, 128])
                pts0, pts1 = {}, {}
                for pr in range(NPAIR):
                    require(("k", pr // 2))
                    for j, pts in ((0, pts0), (1, pts1)):
                        sp = ps_sc.tile([128, 1024], F32, name="sp")
                        for hf in range(2):
                            kc = pr * 2 + hf
                            emit_score(sp, hf, 0, j, 0, kc)
                        pt = probs.tile([128, 1024], BF16, name="pt")
                        nc.scalar.activation(pt[:], sp[:], Act.Exp,
                                             scale=0.125)
                        pts[pr] = pt
                    if pr >= 2:
                        require(("v", (pr - 2) // 2))
                        emit_ctx(0, 0, pr - 2, ctxa0, pts0.pop(pr - 2))
                    pump(RAMP[pr])
                require(("v", NQB - 1))
                emit_ctx(0, 0, NPAIR - 2, ctxa0, pts0.pop(NPAIR - 2))
                emit_ctx(0, 0, NPAIR - 1, ctxa0, pts0.pop(NPAIR - 1))
                pend0 = (0, 0, 0, ctxa0, ctxt)
                cns0 = emit_norm(pend0)
                ctxa1 = ps_ctx.tile([128, NQB, 65], F32, name="ctxa",
                                    padded_shape=[128, NQB, 128])
                for pr in range(NPAIR):
                    emit_ctx(0, 1, pr, ctxa1, pts1.pop(pr))
                pend1 = (0, 1, 0, ctxa1, ctxt)
                cns1 = emit_norm(pend1)
                emit_tp(pend0, cns0)
                return pend1 + (cns1,)

            pend = emit_qb0_interleaved(ctxts[0])
            # j interleaved at qb granularity so dense work (which becomes
            # ready only after a j==1 block) spreads across every qb slot.
            for qb in range(NQB):
                for j in range(2):
                    if qb == 0:
                        continue
                    pend = emit_attention_qb(0, j, qb, ctxts[0], pend, STEADY)
            if dbg:
                st = ostage.tile([128, H], F32, name="dbgq2")
                vflat = vsb[:].rearrange("p a b -> p (a b)")
                for cpart in range(5):
                    w = min(1024, NTC * 130 - cpart * 1024)
                    nc.vector.tensor_copy(st[:, 0:w], vflat[:, cpart * 1024:cpart * 1024 + w])
                    nc.sync.dma_start(dbg["vsb"][:, cpart * 1024:cpart * 1024 + w], st[:, 0:w])
            ctxts[1] = ctxtp.tile([128, S // 128, 128], BF16, name="ctxt1")
            for qb in range(NQB):
                for j in range(2):
                    pend = emit_attention_qb(1, j, qb, ctxts[1], pend, STEADY)
            require(("q", NBLK - 1))
            while fill_jobs:
                fill_jobs.pop(0)[1]()
            emit_tp(pend[:5], pend[5])
            if pend[1] == 1:
                for qc in range(NQB):
                    dense_jobs.append(
                        (426, lambda p=pend[:5], q=qc, r=pend[5][0]:
                         emit_dense_qc(p, q, r, tail=True)))
            while dense_jobs:
                dense_jobs.pop(0)[1]()
            if dbg:
                st2 = ostage.tile([128, H], F32, name="dbgct")
                ctf = ctxts[0][:].rearrange("p a b -> p (a b)")
                for cpart in range(2):
                    nc.vector.tensor_copy(st2[:], ctf[:, cpart * 1024:(cpart + 1) * 1024])
                    nc.sync.dma_start(dbg["ctxt"][:, cpart * 1024:(cpart + 1) * 1024], st2[:])
    nc.compile()
    return nc


def _prepare_inputs(hidden_states, qkv_w, qkv_b, dense_w):
    """Build per-core input maps (host-side slicing/packing, all bf16)."""
    bf16 = ml_dtypes.bfloat16
    x = np.ascontiguousarray(hidden_states, dtype=np.float32).reshape(BS, H)
    xtb = np.ascontiguousarray(x.T).astype(bf16)
    ident = np.eye(128, dtype=bf16)
    qkv_w = np.asarray(qkv_w, dtype=np.float32)
    qkv_b = np.asarray(qkv_b, dtype=np.float32)
    dense_w = np.asarray(dense_w, dtype=np.float32)

    in_maps = []
    m = np.arange(128)
    jj, dd = m // 64, m % 64
    for c in range(NCORES):
        h = 2 * c + jj                      # head index per local dim m
        row_q = h * 192 + dd
        row_k = h * 192 + 64 + dd
        row_v = h * 192 + 128 + dd
        # w?[p, kk, m] = qkv_w[row(m), kk*128 + p]
        wq = np.ascontiguousarray(
            qkv_w[row_q, :].T.reshape(NKK, 128, 128).transpose(1, 0, 2)
        ).astype(bf16)
        wk = np.ascontiguousarray(
            qkv_w[row_k, :].T.reshape(NKK, 128, 128).transpose(1, 0, 2)
        ).astype(bf16)
        wv = np.ascontiguousarray(
            qkv_w[row_v, :].T.reshape(NKK, 128, 128).transpose(1, 0, 2)
        ).astype(bf16)
        # w2m[m, o] = dense_w[o, (2c + m//64)*64 + m%64]
        gcol = h * 64 + dd
        w2m = np.ascontiguousarray(dense_w[:, gcol].T).astype(bf16)
        qb = np.ascontiguousarray(qkv_b[row_q].reshape(128, 1),
                                  dtype=np.float32)
        in_maps.append({
            "xtb": xtb, "wq": wq, "wk": wk, "wv": wv, "w2m": w2m,
            "qbias": qb, "ident": ident,
        })
    return in_maps


def _reference_numpy(hidden_states, attention_mask, qkv_w, qkv_b, dense_w,
                     dense_b):
    """Exact fallback for non-all-ones masks (never hit with spec inputs)."""
    x = np.asarray(hidden_states, dtype=np.float64)
    mask = np.asarray(attention_mask, dtype=np.float64)
    mixed = x @ np.asarray(qkv_w, np.float64).T + np.asarray(qkv_b, np.float64)
    mixed = mixed.reshape(B, S, NH, 3 * HD).transpose(0, 2, 1, 3)
    q, k, v = np.split(mixed, 3, axis=-1)
    scores = np.einsum("bhqd,bhkd->bhqk", q, k) / np.sqrt(HD)
    scores = scores * mask - 10000.0 * (1.0 - mask)
    scores -= scores.max(axis=-1, keepdims=True)
    probs = np.exp(scores)
    probs /= probs.sum(axis=-1, keepdims=True)
    cx = np.einsum("bhqk,bhkd->bhqd", probs, v)
    cx = cx.transpose(0, 2, 1, 3).reshape(B, S, H)
    o = cx @ np.asarray(dense_w, np.float64).T + np.asarray(dense_b, np.float64)
    return o.astype(np.float32)


def _run(inputs, trace=False):
    from concourse.bass_utils import run_bass_kernel_spmd
    if "nc" not in _CACHE:
        _CACHE["nc"] = _build_program()
    nc = _CACHE["nc"]
    in_maps = _prepare_inputs(inputs["hidden_states"], inputs["qkv_w"],
                              inputs["qkv_b"], inputs["dense_w"])
    res = run_bass_kernel_spmd(nc, in_maps, core_ids=list(range(NCORES)),
                               trace=trace)
    partials = np.stack([r["out"] for r in res.results], axis=0)
    full = partials.sum(axis=0, dtype=np.float64)
    qkv_b = np.asarray(inputs["qkv_b"], dtype=np.float64)
    dense_w = np.asarray(inputs["dense_w"], dtype=np.float64)
    g = np.arange(H)
    bv = qkv_b[(g // HD) * 192 + 128 + (g % HD)]
    full += bv @ dense_w.T + np.asarray(inputs["dense_b"], dtype=np.float64)
    return full.astype(np.float32).reshape(B, S, H), res


def kernel(hidden_states, attention_mask, qkv_w, qkv_b, dense_w, dense_b):
    hidden_states = np.asarray(hidden_states)
    attention_mask = np.asarray(attention_mask)
    qkv_w = np.asarray(qkv_w)
    qkv_b = np.asarray(qkv_b)
    dense_w = np.asarray(dense_w)
    dense_b = np.asarray(dense_b)
    if not np.all(attention_mask == 1.0):
        return _reference_numpy(hidden_states, attention_mask, qkv_w, qkv_b,
                                dense_w, dense_b)
    out, _ = _run({
        "hidden_states": hidden_states, "qkv_w": qkv_w, "qkv_b": qkv_b,
        "dense_w": dense_w, "dense_b": dense_b,
    }, trace=bool(int(os.environ.get("KERNEL_TRACE", "0"))))
    return out

